# revision 8
# baseline (speedup 1.0000x reference)
"""Trainium2 Bass kernel for 3x3 same-padded conv (NCHW) scaled by 1/9.

v5: 1D Winograd F(4,3) along W (points {0, 2, -2, 1/2, -1/2}), bf16,
host-side input transform, de-interleaved output layout.

  - Data-parallel over batch: 8 NeuronCores x 4 images each (SPMD).
  - F(4,3): 6 products per 4 outputs -> 2x less PE work than direct conv
    (per-core PE floor 94 us vs 188 us direct bf16).
      f(x) = x^5 - 4.25 x^3 + x
      V_i = BT d (HOST, fp32 -> bf16), U_i = G w (1/9 folded, HOST)
      M_i(y,t) = sum_dy sum_ic U_i(dy) V_i(y+dy, t)      <- PE, PSUM fp32
      Y_a(y,t) = sum_i AT[a,i] M_i(y,t) = out(y, 4t+a)   <- DVE/Act/Pool
  - Points {0,2,-2,.5,-.5} chosen by CPU sweep: rel err 8.3e-3 (vs 1.4e-2
    for classic {0,1,-1,2,-2}); gate 2e-2. AT rows: [1,1,1,1,1,0],
    [0,2,-2,.5,-.5,0], [0,4,4,.25,.25,0], [0,8,-8,.125,-.125,1].
  - Output is written CONTIGUOUSLY as [oc, y, a, t] (Y_a planes); the host
    de-interleaves to [oc, y, 4t+a]. Avoids stride-4 element writes on the
    vector engines (measured 2-4x op slowdown) and keeps DMA contiguous.
  - Engine constraints honored: tensor_tensor reads at most one PSUM input;
    GpSimd cannot access PSUM; TensorScalarPtr ops only on DVE. ScalarE
    stages M1/M3/M5 and applies the power-of-2 output scales.
  - DMA: each image's V is striped across both HWDGE queues in row-pieces
    (img0 finest) so the PE never head-of-line blocks on a tail piece;
    output DMAs alternate queues per chunk.
"""

import numpy as np
import ml_dtypes

import concourse.bacc as bacc
import concourse.mybir as mybir
import concourse.tile as tile
from concourse.bass_utils import run_bass_kernel_spmd

N_CORES = 8
N, IC, H, W = 32, 256, 56, 56
OC, KH, KW = 256, 3, 3
NPC = N // N_CORES
ICT = IC // 128
OCT = OC // 128
HP = H + 2
TQ = 14                     # F(4,3) tiles per row (4 outputs each)
NI = 6
NA = 4
CHUNKS = [(0, 28), (28, 56)]

BF16 = mybir.dt.bfloat16
F32 = mybir.dt.float32
MUL = mybir.AluOpType.mult
ADD = mybir.AluOpType.add

BT = np.array([
    [1, 0, -4.25, 0, 1, 0],
    [0, -0.5, -0.25, 2, 1, 0],
    [0, 0.5, -0.25, -2, 1, 0],
    [0, -2, -4, 0.5, 1, 0],
    [0, 2, -4, -0.5, 1, 0],
    [0, 1, 0, -4.25, 0, 1],
], np.float32)
G = np.array([
    [1, 0, 0],
    [1 / 30, 2 / 30, 4 / 30],
    [1 / 30, -2 / 30, 4 / 30],
    [-8 / 15, -4 / 15, -2 / 15],
    [-8 / 15, 4 / 15, -2 / 15],
    [0, 0, 1],
], np.float32)

_compiled = None


def _build():
    nc = bacc.Bacc("TRN2", target_bir_lowering=False, debug=False,
                   num_devices=N_CORES)

    v_d = nc.dram_tensor("v", [NPC, 128, ICT, NI, HP, TQ], BF16,
                         kind="ExternalInput")
    u_d = nc.dram_tensor("u", [128, OCT, NI, KH, ICT, 128], BF16,
                         kind="ExternalInput")
    o_d = nc.dram_tensor("out", [NPC, OC, H, NA, TQ], F32,
                         kind="ExternalOutput")

    with tile.TileContext(nc) as tc:
        with (
            tc.tile_pool(name="vp", bufs=1) as vpool,
            tc.tile_pool(name="up", bufs=1) as upool,
            tc.tile_pool(name="tp", bufs=4) as tpool,
            tc.tile_pool(name="op", bufs=4) as opool,
            tc.tile_pool(name="ps", bufs=8, space="PSUM") as pspool,
        ):
            usb = upool.tile([128, OCT, NI, KH, ICT, 128], BF16, name="usb")
            vt = []
            for img in range(NPC):
                vt.append(vpool.tile([128, ICT, NI, HP, TQ], BF16,
                                     tag=f"v{img}", name=f"v{img}"))

            # DMA schedule: u oct0 pieces first (needed by the very first
            # matmul), img0's V striped finely across both queues, u oct1
            # next, then imgs 1-3 in halves striped across both queues.
            for i in range(0, NI, 2):
                nc.sync.dma_start(usb[:, 0, i], u_d[:, 0, i])
                nc.scalar.dma_start(usb[:, 0, i + 1], u_d[:, 0, i + 1])
            cuts = [0, 10, 20, 30, 40, 49, HP]
            for k, (a, b) in enumerate(zip(cuts, cuts[1:])):
                q = nc.sync if k % 2 == 0 else nc.scalar
                q.dma_start(vt[0][:, :, :, a:b, :], v_d[0, :, :, :, a:b, :])
            for i in range(0, NI, 2):
                nc.sync.dma_start(usb[:, 1, i], u_d[:, 1, i])
                nc.scalar.dma_start(usb[:, 1, i + 1], u_d[:, 1, i + 1])
            for img in range(1, NPC):
                half = HP // 2
                nc.sync.dma_start(vt[img][:, :, :, :half, :],
                                  v_d[img, :, :, :, :half, :])
                nc.scalar.dma_start(vt[img][:, :, :, half:, :],
                                    v_d[img, :, :, :, half:, :])

            zs = upool.tile([128, 512], BF16, name="zs")
            nc.gpsimd.memset(zs[:], 0.0)
            zp = pspool.tile([128, 512], F32, tag="pt", name="zp")
            for _ in range(12):
                nc.tensor.matmul(zp[:], zs[:, :128], zs[:], start=True,
                                 stop=True)

            ci = 0
            for img in range(NPC):
                for oct_ in range(OCT):
                    for (y0, y1) in CHUNKS:
                        rows = y1 - y0
                        pts = []
                        for i in range(NI):
                            pt = pspool.tile([128, rows, TQ], F32, tag="pt",
                                             name=f"pt{img}_{oct_}_{y0}_{i}")
                            pts.append(pt)
                            for dy in range(KH):
                                for ict in range(ICT):
                                    nc.tensor.matmul(
                                        pt[:],
                                        usb[:, oct_, i, dy, ict],
                                        vt[img][:, ict, i,
                                                y0 + dy:y0 + dy + rows, :],
                                        start=(dy == 0 and ict == 0),
                                        stop=(dy == KH - 1 and ict == ICT - 1),
                                    )
                        ot = opool.tile([128, rows, NA, TQ], F32, tag="ot",
                                        name=f"ot{img}_{oct_}_{y0}")

                        def tp(nm):
                            return tpool.tile([128, rows, TQ], F32, tag=nm,
                                              name=f"{nm}_{img}_{oct_}_{y0}")
                        c1, c3, c5 = tp("c1"), tp("c3"), tp("c5")
                        e, f, p, q = tp("e"), tp("f"), tp("p"), tp("q")
                        a1, t1, t2, t3 = tp("a1"), tp("t1"), tp("t2"), tp("t3")

                        nc.scalar.copy(c1[:], pts[1][:])
                        nc.scalar.copy(c3[:], pts[3][:])
                        nc.scalar.copy(c5[:], pts[5][:])
                        nc.vector.tensor_add(e[:], c1[:], pts[2][:])
                        nc.vector.tensor_sub(f[:], c1[:], pts[2][:])
                        nc.vector.tensor_add(p[:], c3[:], pts[4][:])
                        nc.vector.tensor_sub(q[:], c3[:], pts[4][:])
                        nc.vector.tensor_add(a1[:], e[:], pts[0][:])
                        # Y0 = E + P + M0
                        nc.gpsimd.tensor_add(ot[:, :, 0], a1[:], p[:])
                        # Y1 = (F*4 + Q) * 0.5
                        nc.vector.scalar_tensor_tensor(t1[:], f[:], 4.0, q[:],
                                                       MUL, ADD)
                        nc.scalar.mul(ot[:, :, 1], t1[:], 0.5)
                        # Y2 = (E*16 + P) * 0.25
                        nc.vector.scalar_tensor_tensor(t2[:], e[:], 16.0, p[:],
                                                       MUL, ADD)
                        nc.scalar.mul(ot[:, :, 2], t2[:], 0.25)
                        # Y3 = (Q*0.015625 + F)*8 + M5
                        nc.vector.scalar_tensor_tensor(t3[:], q[:], 0.015625,
                                                       f[:], MUL, ADD)
                        nc.vector.scalar_tensor_tensor(ot[:, :, 3], t3[:],
                                                       8.0, c5[:], MUL, ADD)
                        # Output DMA split across both queues.
                        hr = rows // 2
                        nc.sync.dma_start(
                            o_d[img, oct_ * 128:(oct_ + 1) * 128,
                                y0:y0 + hr], ot[:, :hr])
                        nc.scalar.dma_start(
                            o_d[img, oct_ * 128:(oct_ + 1) * 128,
                                y0 + hr:y1], ot[:, hr:])
                        ci += 1

    nc.compile()
    return nc


def _get_compiled():
    global _compiled
    if _compiled is None:
        _compiled = _build()
    return _compiled


def _prep_inputs(x, w):
    bf = ml_dtypes.bfloat16
    x = np.asarray(x, dtype=np.float32)
    w = np.asarray(w, dtype=np.float32)

    weff = w / (KH * KW)                                  # [oc, ic, dy, kx]
    U = np.stack([sum(G[i, k] * weff[..., k] for k in range(3))
                  for i in range(NI)], axis=2).astype(bf)  # [oc, ic, 6, 3]
    u = np.ascontiguousarray(
        U.reshape(OCT, 128, ICT, 128, NI, KH).transpose(3, 0, 4, 5, 2, 1))

    xp = np.zeros((N, IC, HP, W + 2), np.float32)
    xp[:, :, 1:H + 1, 1:W + 1] = x
    djs = [xp[..., j:j + 4 * (TQ - 1) + 1:4] for j in range(6)]
    V = np.stack([sum(BT[i, j] * djs[j] for j in range(6) if BT[i, j] != 0)
                  for i in range(NI)], axis=2).astype(bf)  # [n, ic, 6, 58, 14]
    v = np.ascontiguousarray(
        V.reshape(N, ICT, 128, NI, HP, TQ).transpose(0, 2, 1, 3, 4, 5))

    return [
        {"v": v[c * NPC:(c + 1) * NPC], "u": u}
        for c in range(N_CORES)
    ]


def kernel(x, w, _trace=False, _trace_kwargs=None):
    nc = _get_compiled()
    in_maps = _prep_inputs(x, w)
    res = run_bass_kernel_spmd(nc, in_maps, list(range(N_CORES)),
                               trace=_trace, **(_trace_kwargs or {}))
    o2 = np.concatenate([res.results[c]["out"] for c in range(N_CORES)],
                        axis=0)                    # [N, OC, H, 4(a), 14(t)]
    # de-interleave: out(y, 4t+a) = o2[y, a, t]
    out = np.ascontiguousarray(
        o2.transpose(0, 1, 2, 4, 3)).reshape(N, OC, H, W)
    if _trace:
        return out, res
    return out
